# revision 1
# baseline (speedup 1.0000x reference)
"""MADE autoregressive sampler on 8 TRN2 NeuronCores.

Strategy:
- Data-parallel over batch: B=4096 -> 512 rows per core; weights replicated.
- Host-side: mask weights (as reference), permute hidden units into
  degree-sorted order -> masked weight matrices become block-triangular at
  128 granularity. At AR step idx only hidden units with degree <= idx-1
  matter (t = ceil(count/128) blocks), so each step runs a truncated,
  block-sparse MLP: ~3x fewer matmul blocks than dense.
- Activations kept transposed [H, B] in SBUF so each layer is a chain of
  psum += WT_block.T @ act_block matmuls (K=128, M=128, N=512).
- Matmul operands bitcast to float32r (full fp32 bits, 1 cycle/row on PE
  for N>=256 vs 4 cycles/row for plain fp32).
"""

import os
import sys
import math
import numpy as np

for _p in ("/opt/trn_rl_repo", "/opt/pypackages"):
    if _p not in sys.path:
        sys.path.insert(0, _p)

import concourse.bass as bass
import concourse.tile as tile
from concourse import bacc
from concourse import mybir
from concourse.bass_utils import run_bass_kernel_spmd

D, H, L, B = 32, 1024, 2, 4096
NCORES = 8
BC = B // NCORES          # 512 batch rows per core
P = 128                   # partitions
NB = H // P               # 8 hidden blocks
F32 = mybir.dt.float32
F32R = mybir.dt.float32r

USE_F32R = os.environ.get("MADE_F32R", "1") == "1"
STOP = int(os.environ.get("MADE_STOP", "32"))
CROSS_OK = os.environ.get("MADE_CROSS", "1") == "1"


def _host_prep(W0, b0, Wh, bh, Wout, bout):
    """Mask, degree-sort, transpose and lay out weights for the device."""
    d_in = np.arange(D)
    d_hid = np.arange(H) % (D - 1)
    d_out = np.arange(D) - 1
    m0 = (d_hid[:, None] >= d_in[None, :]).astype(np.float32)    # [H, D]
    mh = (d_hid[:, None] >= d_hid[None, :]).astype(np.float32)   # [H, H]
    mo = (d_out[:, None] >= d_hid[None, :]).astype(np.float32)   # [D, H]
    mo = np.concatenate([mo, mo], axis=0)                         # [2D, H]

    MW0 = m0 * W0
    MWh0 = mh * Wh[0]
    MWh1 = mh * Wh[1]
    MWo = mo * Wout

    perm = np.argsort(d_hid, kind="stable")
    ds = d_hid[perm]                      # sorted degrees

    W0p = MW0[perm, :]                    # [H, D]
    b0p = b0[perm]
    Wh0p = MWh0[perm][:, perm]            # [H, H] (out, in)
    Wh1p = MWh1[perm][:, perm]
    bh0p = bh[0][perm]
    bh1p = bh[1][perm]
    Wop = MWo[:, perm]                    # [2D, H]

    # lhsT layouts: [K(part), ...]. L1 weights padded K: 32 -> 128.
    W0T = np.zeros((P, H), dtype=np.float32)
    W0T[:D, :] = W0p.T                    # [128, 1024]
    Wh0T = Wh0p.T.reshape(NB, P, H).transpose(1, 0, 2).copy()   # [128, 8, 1024]
    Wh1T = Wh1p.T.reshape(NB, P, H).transpose(1, 0, 2).copy()
    WoT = Wop.T.reshape(NB, P, 2 * D).transpose(1, 0, 2).copy()  # [128, 8, 64]

    b0L = b0p.reshape(NB, P).T.copy()     # [128, 8]
    bh0L = bh0p.reshape(NB, P).T.copy()
    bh1L = bh1p.reshape(NB, P).T.copy()

    # per-step block count: units with degree <= idx-1
    tsteps = []
    for idx in range(D):
        n = int(np.sum(ds <= idx - 1))
        tsteps.append(0 if n == 0 else (n + P - 1) // P)

    # hidden block sparsity (exact, from masked weights)
    nzh0 = np.zeros((NB, NB), dtype=bool)
    nzh1 = np.zeros((NB, NB), dtype=bool)
    for r in range(NB):
        for c in range(NB):
            nzh0[r, c] = np.any(Wh0p[r * P:(r + 1) * P, c * P:(c + 1) * P])
            nzh1[r, c] = np.any(Wh1p[r * P:(r + 1) * P, c * P:(c + 1) * P])

    return dict(W0T=W0T, Wh0T=Wh0T, Wh1T=Wh1T, WoT=WoT,
                b0L=b0L, bh0L=bh0L, bh1L=bh1L, bout=bout.astype(np.float64),
                tsteps=tsteps, nzh0=nzh0, nzh1=nzh1)


def _r(ap):
    return ap


def _build(prep):
    """Build the SPMD Bass program (identical on all cores)."""
    nc = bacc.Bacc("TRN2", target_bir_lowering=False, debug=False,
                   num_devices=NCORES)

    def din(name, shape, dt=F32):
        return nc.dram_tensor(name, list(shape), dt, kind="ExternalInput").ap()

    MMDT = F32R if USE_F32R else F32

    d_w0 = din("w0t", (P, H), MMDT)
    d_wh0 = din("wh0t", (P, NB, H), MMDT)
    d_wh1 = din("wh1t", (P, NB, H), MMDT)
    d_wo = din("wot", (P, NB, 2 * D), MMDT)
    d_b0 = din("b0l", (P, NB))
    d_bh0 = din("bh0l", (P, NB))
    d_bh1 = din("bh1l", (P, NB))
    d_z = din("zb", (1, D, BC))           # [0, i, :] = z row i (partition 0)
    d_bo = din("boutp", (1, 2 * D))       # [0,i]=bout[i]
    d_eye = din("eye", (D, D), MMDT)            # identity, for one-hot scatter lhsT
    d_out = nc.dram_tensor("out", [D, BC], F32, kind="ExternalOutput").ap()

    bout = prep["bout"]
    tsteps, nzh0, nzh1 = prep["tsteps"], prep["nzh0"], prep["nzh1"]

    from contextlib import ExitStack
    with tile.TileContext(nc) as tc, ExitStack() as ctx:
        cp = ctx.enter_context(tc.tile_pool(name="const", bufs=1))
        ps_pool = ctx.enter_context(tc.tile_pool(name="psum", bufs=5, space="PSUM"))
        pso_pool = ctx.enter_context(tc.tile_pool(name="psumo", bufs=1, space="PSUM"))

        w0 = cp.tile([P, H], MMDT, tag="w0")
        wh0 = cp.tile([P, NB, H], MMDT, tag="wh0")
        wh1 = cp.tile([P, NB, H], MMDT, tag="wh1")
        wo = cp.tile([P, NB, 2 * D], MMDT, tag="wo")
        b0s = cp.tile([P, NB], F32, tag="b0s")
        bh0s = cp.tile([P, NB], F32, tag="bh0s")
        bh1s = cp.tile([P, NB], F32, tag="bh1s")
        zb = cp.tile([1, D * BC], F32, tag="zb")
        bos = cp.tile([1, 2 * D], F32, tag="bos")
        oneh = cp.tile([1, D * D], MMDT, tag="oneh")
        xT = cp.tile([P, BC], MMDT, tag="xT")       # rows 0..31 = x.T, rest 0
        a1 = [cp.tile([P, BC], MMDT, tag=f"a1_{r}", name=f"a1_{r}") for r in range(NB)]
        a2 = [cp.tile([P, BC], MMDT, tag=f"a2_{r}", name=f"a2_{r}") for r in range(NB)]
        a3 = [cp.tile([P, BC], MMDT, tag=f"a3_{r}", name=f"a3_{r}") for r in range(NB)]
        es = cp.tile([1, BC], F32, tag="es")       # exp(log_std)
        t2 = cp.tile([1, BC], F32, tag="t2")       # es * z
        xi = cp.tile([1, BC], MMDT, tag="xi")       # new coord, partition 0

        nc.vector.memset(xT.bitcast(F32)[:], 0.0)
        nc.sync.dma_start(b0s[:], d_b0)
        nc.sync.dma_start(bh0s[:], d_bh0)
        nc.sync.dma_start(bh1s[:], d_bh1)
        nc.sync.dma_start(zb[:], d_z)
        nc.sync.dma_start(bos[:], d_bo)
        nc.sync.dma_start(oneh[:], d_eye)
        nc.sync.dma_start(w0[:], d_w0)
        nc.sync.dma_start(wo[:], d_wo)
        for c in range(NB):
            nc.sync.dma_start(wh0[:, c, :], d_wh0[:, c, :])
            nc.sync.dma_start(wh1[:, c, :], d_wh1[:, c, :])

        # step 0: x_0 = z_0 * exp(bout[D]) + bout[0], scattered into xT row 0
        s0 = float(math.exp(bout[D]))
        m0 = float(bout[0])
        nc.vector.tensor_scalar(xi[0:1, :], zb[0:1, 0:BC], s0, m0,
                                mybir.AluOpType.mult, mybir.AluOpType.add)
        ps_s = pso_pool.tile([D, BC], F32, tag="ps_s", name="ps_s0")
        nc.tensor.matmul(ps_s, _r(oneh[0:1, 0:D]), _r(xi[0:1, :]),
                         start=True, stop=True)
        nc.vector.tensor_tensor(xT[0:D, :], xT[0:D, :], ps_s,
                                mybir.AluOpType.add)

        for idx in range(1, STOP):
            t = tsteps[idx]
            # ---- layer 1: a1[r] = relu(W0T[:,rblk].T @ xT + b0)
            for r in range(t):
                ps = ps_pool.tile([P, BC], F32, tag="ps")
                nc.tensor.matmul(ps, _r(w0[:, r * P:(r + 1) * P]), _r(xT[:]),
                                 start=True, stop=True)
                nc.scalar.activation(a1[r][:], ps,
                                     mybir.ActivationFunctionType.Relu,
                                     bias=b0s[:, r:r + 1], scale=1.0)
            # ---- hidden layer 2
            for r in range(t):
                cols = [c for c in range(t) if nzh0[r, c]]
                ps = ps_pool.tile([P, BC], F32, tag="ps")
                for j, c in enumerate(cols):
                    nc.tensor.matmul(ps, _r(wh0[:, c, r * P:(r + 1) * P]),
                                     _r(a1[c][:]),
                                     start=(j == 0), stop=(j == len(cols) - 1))
                nc.scalar.activation(a2[r][:], ps,
                                     mybir.ActivationFunctionType.Relu,
                                     bias=bh0s[:, r:r + 1], scale=1.0)
            # ---- hidden layer 3
            for r in range(t):
                cols = [c for c in range(t) if nzh1[r, c]]
                ps = ps_pool.tile([P, BC], F32, tag="ps")
                for j, c in enumerate(cols):
                    nc.tensor.matmul(ps, _r(wh1[:, c, r * P:(r + 1) * P]),
                                     _r(a2[c][:]),
                                     start=(j == 0), stop=(j == len(cols) - 1))
                nc.scalar.activation(a3[r][:], ps,
                                     mybir.ActivationFunctionType.Relu,
                                     bias=bh1s[:, r:r + 1], scale=1.0)
            # ---- out layer: two M=1 groups, both land at partition 0
            pmu = pso_pool.tile([1, BC], F32, tag="pmu", name=f"pmu{idx}")
            pls = pso_pool.tile([1, BC], F32, tag="pls", name=f"pls{idx}")
            for j in range(t):
                nc.tensor.matmul(pls, _r(wo[:, j, idx + D:idx + D + 1]),
                                 _r(a3[j][:]),
                                 start=(j == 0), stop=(j == t - 1))
            for j in range(t):
                nc.tensor.matmul(pmu, _r(wo[:, j, idx:idx + 1]),
                                 _r(a3[j][:]),
                                 start=(j == 0), stop=(j == t - 1))
            # es = exp(ls + bls); t2 = es * z_idx; xi = (mu + bmu) + t2
            nc.scalar.activation(es[0:1, :], pls[0:1, :],
                                 mybir.ActivationFunctionType.Exp,
                                 bias=bos[0:1, idx + D:idx + D + 1], scale=1.0)
            nc.vector.tensor_tensor(t2[0:1, :], es[0:1, :], zb[0:1, idx * BC:(idx + 1) * BC],
                                    mybir.AluOpType.mult)
            nc.vector.scalar_tensor_tensor(xi[0:1, :], pmu[0:1, :],
                                           bos[0:1, idx:idx + 1], t2[0:1, :],
                                           mybir.AluOpType.add,
                                           mybir.AluOpType.add)
            # scatter xi into xT row idx: rank-1 one-hot matmul + add
            ps_s = pso_pool.tile([D, BC], F32, tag="ps_s", name=f"ps_s{idx}")
            nc.tensor.matmul(ps_s, _r(oneh[0:1, idx * D:(idx + 1) * D]), _r(xi[0:1, :]),
                             start=True, stop=True)
            nc.vector.tensor_tensor(xT[0:D, :], xT[0:D, :], ps_s,
                                    mybir.AluOpType.add)

        nc.sync.dma_start(d_out, xT[:D, :].bitcast(F32) if USE_F32R else xT[:D, :])

    nc.compile()
    return nc


_CACHE = {}


def _get_program(prep):
    if "nc" not in _CACHE:
        _CACHE["nc"] = _build(prep)
    return _CACHE["nc"]


def _run(inputs, trace=False):
    z = np.asarray(inputs["z"], dtype=np.float32)
    prep = _host_prep(np.asarray(inputs["W0"], np.float32),
                      np.asarray(inputs["b0"], np.float32),
                      np.asarray(inputs["Wh"], np.float32),
                      np.asarray(inputs["bh"], np.float32),
                      np.asarray(inputs["Wout"], np.float32),
                      np.asarray(inputs["bout"], np.float32))
    nc = _get_program(prep)

    in_maps = []
    for c in range(NCORES):
        zs = z[c * BC:(c + 1) * BC, :]                 # [512, 32]
        zbuf = np.ascontiguousarray(zs.T.reshape(1, D * BC))
        in_maps.append({
            "w0t": prep["W0T"], "wh0t": prep["Wh0T"], "wh1t": prep["Wh1T"],
            "wot": prep["WoT"], "b0l": prep["b0L"], "bh0l": prep["bh0L"],
            "bh1l": prep["bh1L"], "zb": zbuf,
            "boutp": prep["bout"].astype(np.float32)[None, :],
            "eye": np.eye(D, dtype=np.float32).reshape(1, D * D),
        })

    res = run_bass_kernel_spmd(nc, in_maps, core_ids=list(range(NCORES)),
                               trace=trace)
    out = np.empty((B, D), dtype=np.float32)
    for c in range(NCORES):
        out[c * BC:(c + 1) * BC, :] = res.results[c]["out"].T
    return out, res


def kernel(**inputs):
    out, _ = _run(inputs, trace=False)
    return out



# revision 22
# speedup vs baseline: 2.3544x; 2.3544x over previous
"""MADE autoregressive sampler on 8 TRN2 NeuronCores — incremental frontier.

Strategy (vs. the full-recompute baseline):
- Data-parallel over batch: B=4096 -> 512 rows per core; weights replicated.
- Degree-sort hidden units. In MADE, a hidden unit's activation is FINAL once
  x columns 0..deg are set, so per AR step only the 1-2 "frontier" blocks
  (those containing degree idx-1) need recomputation. Everything else is
  computed once and cached:
    * z1 (layer-1 preact) kept in PSUM, updated by a rank-1 matmul per step.
    * S2/S3 = frozen off-diagonal partial sums per frontier block, cached in
      SBUF and restored into PSUM each step (then diag matmul accumulates).
    * theta (output-layer contributions of finalized blocks) accumulates in
      one PSUM bank, in batch-major chunk layout so the per-step tail ops are
      [128, 4] instead of [1, 512].
- fp16 operands everywhere (fp32 PSUM accumulation).
- Elementwise load spread across Scalar(Act)/Vector(DVE)/Pool(gpsimd).
"""

import os
import sys
import math
import hashlib
import numpy as np

for _p in ("/opt/trn_rl_repo", "/opt/pypackages"):
    if _p not in sys.path:
        sys.path.insert(0, _p)

import concourse.bass as bass
import concourse.tile as tile
from concourse import bacc
from concourse import mybir
from concourse.bass_utils import run_bass_kernel_spmd

D, H, L, B = 32, 1024, 2, 4096
NCORES = 8
BC = B // NCORES          # 512 batch rows per core
P = 128
NB = H // P               # 8 hidden blocks
NJ = BC // P              # 4 batch chunks of 128
F32 = mybir.dt.float32
F16 = mybir.dt.float16

DTYPE = os.environ.get("MADE_DTYPE", "fp16")
MMDT = {"fp16": mybir.dt.float16, "bf16": mybir.dt.bfloat16,
        "f32r": mybir.dt.float32r}[DTYPE]
NPDT = {"fp16": np.float16, "bf16": np.float32, "f32r": np.float32}[DTYPE]
STOP = int(os.environ.get("MADE_STOP", "32"))

AluOp = mybir.AluOpType
ActFn = mybir.ActivationFunctionType


def _schedule():
    """Static per-step schedule from the degree structure."""
    d_hid = np.arange(H) % (D - 1)
    perm = np.argsort(d_hid, kind="stable")
    ds = d_hid[perm]
    g_lo = [int(ds[P * b]) for b in range(NB)]
    g_hi = [int(ds[P * b + P - 1]) for b in range(NB)]
    entry = [g_lo[b] + 1 for b in range(NB)]
    final = [g_hi[b] + 1 for b in range(NB)]
    return perm, ds, g_lo, g_hi, entry, final


def _host_prep(W0, b0, Wh, bh, Wout, bout):
    d_in = np.arange(D)
    d_hid = np.arange(H) % (D - 1)
    d_out = np.arange(D) - 1
    m0 = (d_hid[:, None] >= d_in[None, :]).astype(np.float32)
    mh = (d_hid[:, None] >= d_hid[None, :]).astype(np.float32)
    mo = (d_out[:, None] >= d_hid[None, :]).astype(np.float32)
    mo = np.concatenate([mo, mo], axis=0)

    perm, ds, g_lo, g_hi, entry, final = _schedule()

    W0p = (m0 * W0)[perm]                     # [H, D]
    Wh0p = (mh * Wh[0])[perm][:, perm]        # [H, H] (out, in)
    Wh1p = (mh * Wh[1])[perm][:, perm]
    Wop = (mo * Wout)[:, perm]                # [2D, H]
    b0p = b0[perm]
    bh0p = bh[0][perm]
    bh1p = bh[1][perm]

    # lhsT layouts
    W0T = np.ascontiguousarray(W0p.T).astype(NPDT)          # [32, H]
    Wh0T = np.ascontiguousarray(
        Wh0p.T.reshape(NB, P, H).transpose(1, 0, 2)).astype(NPDT)  # [128, NB, H]
    Wh1T = np.ascontiguousarray(
        Wh1p.T.reshape(NB, P, H).transpose(1, 0, 2)).astype(NPDT)
    # interleaved output weights: col 2i = mu_i, col 2i+1 = ls_i
    WoI = np.empty((H, 2 * D), dtype=np.float32)
    WoI[:, 0::2] = Wop[:D, :].T
    WoI[:, 1::2] = Wop[D:, :].T
    WoIT = np.ascontiguousarray(
        WoI.reshape(NB, P, 2 * D).transpose(1, 0, 2)).astype(NPDT)  # [128, NB, 64]

    b0L = np.ascontiguousarray(b0p.reshape(NB, P).T).astype(np.float32)
    bh0L = np.ascontiguousarray(bh0p.reshape(NB, P).T).astype(np.float32)
    bh1L = np.ascontiguousarray(bh1p.reshape(NB, P).T).astype(np.float32)

    nzh0 = np.zeros((NB, NB), dtype=bool)
    nzh1 = np.zeros((NB, NB), dtype=bool)
    for r in range(NB):
        for c in range(NB):
            nzh0[r, c] = bool(np.any(Wh0p[r * P:(r + 1) * P, c * P:(c + 1) * P]))
            nzh1[r, c] = bool(np.any(Wh1p[r * P:(r + 1) * P, c * P:(c + 1) * P]))

    return dict(W0T=W0T, Wh0T=Wh0T, Wh1T=Wh1T, WoIT=WoIT,
                b0L=b0L, bh0L=bh0L, bh1L=bh1L,
                bout=bout.astype(np.float64),
                nzh0=nzh0, nzh1=nzh1,
                g_lo=g_lo, g_hi=g_hi, entry=entry, final=final)


def _build(prep):
    nc = bacc.Bacc("TRN2", target_bir_lowering=False, debug=False,
                   num_devices=NCORES)

    def din(name, shape, dt=F32):
        return nc.dram_tensor(name, list(shape), dt, kind="ExternalInput").ap()

    d_w0 = din("w0t", (D, H), MMDT)
    d_w0r = din("w0r", (1, D, H), MMDT)
    d_wh0 = din("wh0t", (P, NB, H), MMDT)
    d_wh1 = din("wh1t", (P, NB, H), MMDT)
    d_wo = din("wot", (P, NB, 2 * D), MMDT)
    d_b0 = din("b0l", (P, NB))
    d_bh0 = din("bh0l", (P, NB))
    d_bh1 = din("bh1l", (P, NB))
    d_z = din("zb", (P, NJ * D), MMDT)       # batch-major [p, j*32+i]
    d_eye = din("eye", (P, P), MMDT)
    d_out = nc.dram_tensor("out", [P, NJ * D], F32, kind="ExternalOutput").ap()

    bout = prep["bout"]
    nzh0, nzh1 = prep["nzh0"], prep["nzh1"]
    g_lo, g_hi = prep["g_lo"], prep["g_hi"]
    entry, final = prep["entry"], prep["final"]

    def active_at(idx):
        return [b for b in range(NB) if g_lo[b] <= idx - 1 <= g_hi[b]]

    from contextlib import ExitStack
    with tile.TileContext(nc) as tc, ExitStack() as ctx:
        cp = ctx.enter_context(tc.tile_pool(name="const", bufs=1))
        pp = ctx.enter_context(tc.tile_pool(name="psum", bufs=1, space="PSUM"))

        # ---- PSUM: exactly 8 banks ----
        pz1 = [pp.tile([P, BC], F32, tag=f"pz1_{i}", name=f"pz1_{i}") for i in range(2)]
        pz2 = [pp.tile([P, BC], F32, tag=f"pz2_{i}", name=f"pz2_{i}") for i in range(2)]
        pz3 = [pp.tile([P, BC], F32, tag=f"pz3_{i}", name=f"pz3_{i}") for i in range(2)]
        pth = pp.tile([P, NJ, 2 * D], F32, tag="pth", name="pth")   # theta (chunk-major)
        pmisc = pp.tile([P, 256, 2], F32, tag="pmisc", name="pmisc")
        # pmisc map: [:, 0:4, 0] mu-frontier, [:, 0:4, 1] ls-frontier,
        #            [0:4, 64:128, :] xiT, [0:32, 128:192, :] / [0:32, 192:256, :] xB^T ping/pong

        # ---- SBUF ----
        w0 = cp.tile([D, H], MMDT, tag="w0")
        w0r = cp.tile([1, D, H], MMDT, tag="w0r")
        wh0 = cp.tile([P, NB, H], MMDT, tag="wh0")
        wh1 = cp.tile([P, NB, H], MMDT, tag="wh1")
        wo = cp.tile([P, NB, 2 * D], MMDT, tag="wo")
        eye = cp.tile([P, P], MMDT, tag="eye")
        b0s = cp.tile([P, NB], F32, tag="b0s")
        bh0s = cp.tile([P, NB], F32, tag="bh0s")
        bh1s = cp.tile([P, NB], F32, tag="bh1s")
        zB = cp.tile([P, NJ, D], MMDT, tag="zB")
        xB = cp.tile([P, NJ, D], MMDT, tag="xB")
        xBf = cp.tile([P, NJ * D], F32, tag="xBf")
        xT4 = cp.tile([D, NJ, P], MMDT, tag="xT4")
        thetaS = cp.tile([P, NJ, 2 * D], F32, tag="thetaS")
        xiB = cp.tile([P, NJ], MMDT, tag="xiB")
        xiT = cp.tile([1, NJ, P], MMDT, tag="xiT")
        uls = cp.tile([P, NJ], F32, tag="uls")
        umu = cp.tile([P, NJ], MMDT, tag="umu")
        es = cp.tile([P, NJ], MMDT, tag="es")
        t2 = cp.tile([P, NJ], MMDT, tag="t2")
        a1 = [cp.tile([P, BC], MMDT, tag=f"a1_{r}", name=f"a1_{r}") for r in range(NB)]
        a2 = [cp.tile([P, BC], MMDT, tag=f"a2_{r}", name=f"a2_{r}") for r in range(NB)]
        a3 = [cp.tile([P, BC], MMDT, tag=f"a3_{r}", name=f"a3_{r}") for r in range(NB)]
        S2sb = [cp.tile([P, BC], F32, tag=f"s2_{i}", name=f"s2_{i}") for i in range(2)]
        S3sb = [cp.tile([P, BC], F32, tag=f"s3_{i}", name=f"s3_{i}") for i in range(2)]

        # ---- DMA in (order matters: early-needed first) ----
        nc.sync.dma_start(zB[:], d_z)
        nc.sync.dma_start(b0s[:], d_b0)
        nc.sync.dma_start(bh0s[:], d_bh0)
        nc.sync.dma_start(bh1s[:], d_bh1)
        nc.sync.dma_start(w0[:], d_w0)
        nc.sync.dma_start(w0r[:], d_w0r)
        nc.sync.dma_start(eye[:], d_eye)
        nc.sync.dma_start(wo[:], d_wo)
        for c in range(NB):
            nc.sync.dma_start(wh0[:, c, :], d_wh0[:, c, :])
            nc.sync.dma_start(wh1[:, c, :], d_wh1[:, c, :])

        nc.vector.memset(xB[:], 0.0)

        assert MMDT in (F16, mybir.dt.bfloat16), "only 2-byte matmul dtypes"
        # per-chunk [1, 128] transpose landing pads, contiguous -> [1, 512]
        xiTv = [pmisc[0:1, 4 + 32 * j:36 + 32 * j, :].bitcast(MMDT)
                for j in range(NJ)]
        xiTfull = pmisc[0:1, 4:132, :].bitcast(MMDT)         # [1, 128, 4] = 512
        xtt = [pmisc[0:D, 132:164, :].bitcast(MMDT),
               pmisc[0:D, 164:196, :].bitcast(MMDT)]

        def xi_transpose():
            for j in range(NJ):
                nc.tensor.transpose(xiTv[j], xiB[:, j:j + 1], eye[:])
            nc.vector.tensor_scalar_add(xiT[:], xiTfull, 0.0)

        def mm(out, lhsT, rhs, start, stop):
            nc.tensor.matmul(out, lhsT, rhs, start=start, stop=stop,
                             skip_group_check=True)

        def relu_split(dst, psrc, bias_col):
            """dst = relu(psrc + bias) split across Act / DVE (Pool can't
            touch PSUM)."""
            h = 256
            nc.scalar.activation(dst[:, 0:h], psrc[:, 0:h], ActFn.Relu,
                                 bias=bias_col, scale=1.0)
            nc.vector.tensor_scalar(dst[:, h:BC], psrc[:, h:BC],
                                    bias_col, 0.0, AluOp.add, AluOp.max)

        def restore_split(pdst, ssrc):
            """psum <- sbuf copy split across DVE / Act."""
            h = 256
            nc.vector.tensor_scalar_add(pdst[:, 0:h], ssrc[:, 0:h], 0.0)
            nc.scalar.copy(pdst[:, h:BC], ssrc[:, h:BC])

        # ---- step 0: x_0 = z_0 * exp(bout[D]) + bout[0] ----
        s0 = float(math.exp(bout[D]))
        m0c = float(bout[0])
        nc.vector.tensor_scalar(xiB[:], zB[:, :, 0], s0, m0c,
                                AluOp.mult, AluOp.add)
        nc.gpsimd.tensor_scalar_add(xB[:, :, 0], xiB[:], 0.0)
        # xiT for step 1's rank-1
        xi_transpose()

        S2ready = [False] * NB
        S3ready = [False] * NB
        theta_init = [False] * NJ
        pf_par = 0

        for idx in range(1, STOP):
            act_blocks = active_at(idx)
            entering = [b for b in act_blocks if entry[b] == idx]
            finalizing = [b for b in act_blocks if final[b] == idx]

            # -- 1. prefetch z1 for a block entering NEXT step (reads xB
            #       BEFORE this step's scatter; program order enforces WAR) --
            pre = [b for b in range(1, NB) if entry[b] == idx + 1]
            for b in pre:
                for j in range(NJ):
                    pg = xtt[j % 2]
                    nc.tensor.transpose(pg, xB[:, j, :], eye[:])
                    nc.vector.tensor_scalar_add(xT4[:, j, :], pg, 0.0)
                mm(pz1[b % 2][:], w0[:, b * P:(b + 1) * P], xT4[:], True, True)

            # -- 2. restore z2/z3 from cached S --
            for b in act_blocks:
                if S2ready[b]:
                    restore_split(pz2[b % 2], S2sb[b % 2])
                if S3ready[b]:
                    restore_split(pz3[b % 2], S3sb[b % 2])

            # -- 3. rank-1 z1 update (col idx-1) for all active blocks --
            for b in act_blocks:
                first = (b == 0 and idx == 1)
                mm(pz1[b % 2][:],
                   w0r[0:1, idx - 1, b * P:(b + 1) * P],
                   xiT[:], first, True)

            # -- 4. relu1 --
            for b in act_blocks:
                relu_split(a1[b], pz1[b % 2], b0s[:, b:b + 1])

            # -- 5. layer 2 --
            for b in act_blocks:
                cols = [c for c in range(NB)
                        if nzh0[b, c] and g_lo[c] <= idx - 1]
                if S2ready[b]:
                    todo = [c for c in cols if c >= b]
                    for k, c in enumerate(todo):
                        mm(pz2[b % 2][:], wh0[:, c, b * P:(b + 1) * P],
                           a1[c][:], False, k == len(todo) - 1)
                else:
                    for k, c in enumerate(cols):
                        mm(pz2[b % 2][:], wh0[:, c, b * P:(b + 1) * P],
                           a1[c][:], k == 0, k == len(cols) - 1)
                relu_split(a2[b], pz2[b % 2], bh0s[:, b:b + 1])

            # -- 6. layer 3 --
            for b in act_blocks:
                cols = [c for c in range(NB)
                        if nzh1[b, c] and g_lo[c] <= idx - 1]
                if S3ready[b]:
                    todo = [c for c in cols if c >= b]
                    for k, c in enumerate(todo):
                        mm(pz3[b % 2][:], wh1[:, c, b * P:(b + 1) * P],
                           a2[c][:], False, k == len(todo) - 1)
                else:
                    for k, c in enumerate(cols):
                        mm(pz3[b % 2][:], wh1[:, c, b * P:(b + 1) * P],
                           a2[c][:], k == 0, k == len(cols) - 1)
                relu_split(a3[b], pz3[b % 2], bh1s[:, b:b + 1])

            # -- 7. frontier output contribution (batch-major, N=2) --
            for j in range(NJ):
                for k, b in enumerate(act_blocks):
                    mm(pmisc[:, j, 0:2],
                       a3[b][:, j * P:(j + 1) * P],
                       wo[:, b, 2 * idx:2 * idx + 2],
                       k == 0, k == len(act_blocks) - 1)

            # -- 8. tail --
            bmu = float(bout[idx])
            bls = float(bout[idx + D])
            theta_ok = idx > final[0]
            if theta_ok:
                nc.vector.scalar_tensor_tensor(
                    uls[:], pmisc[:, 0:NJ, 1], bls, thetaS[:, :, 2 * idx + 1],
                    AluOp.add, AluOp.add)
                nc.vector.scalar_tensor_tensor(
                    umu[:], pmisc[:, 0:NJ, 0], bmu, thetaS[:, :, 2 * idx],
                    AluOp.add, AluOp.add)
            else:
                nc.vector.tensor_scalar_add(uls[:], pmisc[:, 0:NJ, 1], bls)
                nc.vector.tensor_scalar_add(umu[:], pmisc[:, 0:NJ, 0], bmu)
            nc.scalar.activation(es[:], uls[:], ActFn.Exp)
            nc.gpsimd.tensor_tensor(t2[:], es[:], zB[:, :, idx], AluOp.mult)
            nc.gpsimd.tensor_tensor(xiB[:], t2[:], umu[:], AluOp.add)

            # -- 9. scatter + transpose for next step --
            nc.gpsimd.tensor_scalar_add(xB[:, :, idx], xiB[:], 0.0)
            if idx < STOP - 1:
                xi_transpose()

            # -- 10. finalize theta (after tail reads of pth) --
            for b in finalizing:
                if idx >= STOP - 1:
                    continue
                for j in range(NJ):
                    mm(pth[:, j, :],
                       a3[b][:, j * P:(j + 1) * P],
                       wo[:, b, :],
                       not theta_init[j], True)
                    theta_init[j] = True
                nc.vector.tensor_scalar_add(thetaS[:], pth[:], 0.0)

            # -- 11. cache S2/S3 one step after entry --
            for b in act_blocks:
                if b > 0 and not S2ready[b] and idx == entry[b] + 1 \
                        and idx < final[b]:
                    sc = (b - 1) % 2
                    cc = [c for c in range(NB) if nzh0[b, c] and c < b]
                    for k, c in enumerate(cc):
                        mm(pz2[sc][:], wh0[:, c, b * P:(b + 1) * P],
                           a1[c][:], k == 0, k == len(cc) - 1)
                    nc.vector.tensor_scalar_add(S2sb[b % 2][:], pz2[sc][:], 0.0)
                    S2ready[b] = True
                    cc3 = [c for c in range(NB) if nzh1[b, c] and c < b]
                    for k, c in enumerate(cc3):
                        mm(pz3[sc][:], wh1[:, c, b * P:(b + 1) * P],
                           a2[c][:], k == 0, k == len(cc3) - 1)
                    nc.vector.tensor_scalar_add(S3sb[b % 2][:], pz3[sc][:], 0.0)
                    S3ready[b] = True
            for b in list(range(NB)):
                if final[b] == idx:
                    S2ready[b] = False
                    S3ready[b] = False

        # ---- output ----
        nc.scalar.copy(xBf[:], xB[:])
        nc.sync.dma_start(d_out, xBf[:])

    nc.compile()
    return nc


_CACHE = {}


def _get_program(prep):
    key = (DTYPE, STOP, hashlib.md5(prep["bout"].tobytes()).hexdigest())
    if key not in _CACHE:
        _CACHE[key] = _build(prep)
    return _CACHE[key]


def _run(inputs, trace=False):
    z = np.asarray(inputs["z"], dtype=np.float32)
    prep = _host_prep(np.asarray(inputs["W0"], np.float32),
                      np.asarray(inputs["b0"], np.float32),
                      np.asarray(inputs["Wh"], np.float32),
                      np.asarray(inputs["bh"], np.float32),
                      np.asarray(inputs["Wout"], np.float32),
                      np.asarray(inputs["bout"], np.float32))
    nc = _get_program(prep)

    eye = np.eye(P, dtype=NPDT)
    in_maps = []
    for c in range(NCORES):
        zs = z[c * BC:(c + 1) * BC, :]                     # [512, 32]
        # batch-major: [p, j, i] = z[j*128+p, i]
        zb = np.ascontiguousarray(
            zs.reshape(NJ, P, D).transpose(1, 0, 2).reshape(P, NJ * D)
        ).astype(NPDT)
        in_maps.append({
            "w0t": prep["W0T"], "wh0t": prep["Wh0T"], "wh1t": prep["Wh1T"],
            "wot": prep["WoIT"], "b0l": prep["b0L"], "bh0l": prep["bh0L"],
            "bh1l": prep["bh1L"], "zb": zb, "eye": eye,
            "w0r": prep["W0T"].reshape(1, D, H),
        })

    res = run_bass_kernel_spmd(nc, in_maps, core_ids=list(range(NCORES)),
                               trace=trace)
    out = np.empty((B, D), dtype=np.float32)
    for c in range(NCORES):
        buf = res.results[c]["out"]                        # [128, 128]
        out[c * BC:(c + 1) * BC, :] = (
            buf.reshape(P, NJ, D).transpose(1, 0, 2).reshape(BC, D))
    return out, res


def kernel(**inputs):
    out, _ = _run(inputs, trace=False)
    return out


# revision 27
# speedup vs baseline: 2.6140x; 1.1103x over previous
"""MADE autoregressive sampler on 8 TRN2 NeuronCores — incremental frontier.

Strategy (vs. the full-recompute baseline):
- Data-parallel over batch: B=4096 -> 512 rows per core; weights replicated.
- Degree-sort hidden units. In MADE, a hidden unit's activation is FINAL once
  x columns 0..deg are set, so per AR step only the 1-2 "frontier" blocks
  (those containing degree idx-1) need recomputation. Everything else is
  computed once and cached:
    * z1 (layer-1 preact) kept in PSUM, updated by a rank-1 matmul per step.
    * S2/S3 = frozen off-diagonal partial sums per frontier block, cached in
      SBUF and restored into PSUM each step (then diag matmul accumulates).
    * theta (output-layer contributions of finalized blocks) accumulates in
      one PSUM bank, in batch-major chunk layout so the per-step tail ops are
      [128, 4] instead of [1, 512].
- fp16 operands everywhere (fp32 PSUM accumulation).
- Elementwise load spread across Scalar(Act)/Vector(DVE)/Pool(gpsimd).
"""

import os
import sys
import math
import hashlib
import numpy as np

for _p in ("/opt/trn_rl_repo", "/opt/pypackages"):
    if _p not in sys.path:
        sys.path.insert(0, _p)

import concourse.bass as bass
import concourse.tile as tile
from concourse import bacc
from concourse import mybir
from concourse.bass_utils import run_bass_kernel_spmd

D, H, L, B = 32, 1024, 2, 4096
NCORES = 8
BC = B // NCORES          # 512 batch rows per core
P = 128
NB = H // P               # 8 hidden blocks
NJ = BC // P              # 4 batch chunks of 128
F32 = mybir.dt.float32
F16 = mybir.dt.float16

DTYPE = os.environ.get("MADE_DTYPE", "fp16")
MMDT = {"fp16": mybir.dt.float16, "bf16": mybir.dt.bfloat16,
        "f32r": mybir.dt.float32r}[DTYPE]
NPDT = {"fp16": np.float16, "bf16": np.float32, "f32r": np.float32}[DTYPE]
STOP = int(os.environ.get("MADE_STOP", "32"))

AluOp = mybir.AluOpType
ActFn = mybir.ActivationFunctionType


def _schedule():
    """Static per-step schedule from the degree structure."""
    d_hid = np.arange(H) % (D - 1)
    perm = np.argsort(d_hid, kind="stable")
    ds = d_hid[perm]
    g_lo = [int(ds[P * b]) for b in range(NB)]
    g_hi = [int(ds[P * b + P - 1]) for b in range(NB)]
    entry = [g_lo[b] + 1 for b in range(NB)]
    final = [g_hi[b] + 1 for b in range(NB)]
    return perm, ds, g_lo, g_hi, entry, final


def _host_prep(W0, b0, Wh, bh, Wout, bout):
    d_in = np.arange(D)
    d_hid = np.arange(H) % (D - 1)
    d_out = np.arange(D) - 1
    m0 = (d_hid[:, None] >= d_in[None, :]).astype(np.float32)
    mh = (d_hid[:, None] >= d_hid[None, :]).astype(np.float32)
    mo = (d_out[:, None] >= d_hid[None, :]).astype(np.float32)
    mo = np.concatenate([mo, mo], axis=0)

    perm, ds, g_lo, g_hi, entry, final = _schedule()

    W0p = (m0 * W0)[perm]                     # [H, D]
    Wh0p = (mh * Wh[0])[perm][:, perm]        # [H, H] (out, in)
    Wh1p = (mh * Wh[1])[perm][:, perm]
    Wop = (mo * Wout)[:, perm]                # [2D, H]
    b0p = b0[perm]
    bh0p = bh[0][perm]
    bh1p = bh[1][perm]

    # lhsT layouts
    W0T = np.ascontiguousarray(W0p.T).astype(NPDT)          # [32, H]
    Wh0T = np.ascontiguousarray(
        Wh0p.T.reshape(NB, P, H).transpose(1, 0, 2)).astype(NPDT)  # [128, NB, H]
    Wh1T = np.ascontiguousarray(
        Wh1p.T.reshape(NB, P, H).transpose(1, 0, 2)).astype(NPDT)
    # interleaved output weights: col 2i = mu_i, col 2i+1 = ls_i
    WoI = np.empty((H, 2 * D), dtype=np.float32)
    WoI[:, 0::2] = Wop[:D, :].T
    WoI[:, 1::2] = Wop[D:, :].T
    WoIT = np.ascontiguousarray(
        WoI.reshape(NB, P, 2 * D).transpose(1, 0, 2)).astype(NPDT)  # [128, NB, 64]

    b0L = np.ascontiguousarray(b0p.reshape(NB, P).T).astype(np.float32)
    bh0L = np.ascontiguousarray(bh0p.reshape(NB, P).T).astype(np.float32)
    bh1L = np.ascontiguousarray(bh1p.reshape(NB, P).T).astype(np.float32)

    nzh0 = np.zeros((NB, NB), dtype=bool)
    nzh1 = np.zeros((NB, NB), dtype=bool)
    for r in range(NB):
        for c in range(NB):
            nzh0[r, c] = bool(np.any(Wh0p[r * P:(r + 1) * P, c * P:(c + 1) * P]))
            nzh1[r, c] = bool(np.any(Wh1p[r * P:(r + 1) * P, c * P:(c + 1) * P]))

    return dict(W0T=W0T, Wh0T=Wh0T, Wh1T=Wh1T, WoIT=WoIT,
                b0L=b0L, bh0L=bh0L, bh1L=bh1L,
                bout=bout.astype(np.float64),
                nzh0=nzh0, nzh1=nzh1,
                g_lo=g_lo, g_hi=g_hi, entry=entry, final=final)


def _build(prep):
    nc = bacc.Bacc("TRN2", target_bir_lowering=False, debug=False,
                   num_devices=NCORES)

    def din(name, shape, dt=F32):
        return nc.dram_tensor(name, list(shape), dt, kind="ExternalInput").ap()

    HB = BC // 2                                # 256: batch half per side
    d_w0 = din("w0t", (D, H), MMDT)
    d_w0r = din("w0r", (1, D, H), MMDT)
    d_wh0 = din("wh0t", (P, NB, H), MMDT)
    d_wh1 = din("wh1t", (P, NB, H), MMDT)
    d_wo = din("wot", (P, NB, 2 * D), MMDT)
    d_b0 = din("b0l", (P, NB))
    d_bh0 = din("bh0l", (P, NB))
    d_bh1 = din("bh1l", (P, NB))
    d_z = din("zb", (P, NJ * D), MMDT)          # batch-major [p, j*32+i]
    d_bml = din("bml", (P, NJ * 2 * D))         # bout replicated, interleaved
    d_eye = din("eye", (P, P), MMDT)
    d_out = nc.dram_tensor("out", [P, NJ * D], F32, kind="ExternalOutput").ap()

    bout = prep["bout"]
    nzh0, nzh1 = prep["nzh0"], prep["nzh1"]
    g_lo, g_hi = prep["g_lo"], prep["g_hi"]
    entry, final = prep["entry"], prep["final"]

    def active_at(idx):
        return [b for b in range(NB) if g_lo[b] <= idx - 1 <= g_hi[b]]

    from contextlib import ExitStack
    with tile.TileContext(nc) as tc, ExitStack() as ctx:
        cp = ctx.enter_context(tc.tile_pool(name="const", bufs=1))
        pp = ctx.enter_context(tc.tile_pool(name="psum", bufs=1, space="PSUM"))

        # ---- PSUM: exactly 8 banks (L = batch cols 0:256 -> Act side,
        #      R = cols 256:512 -> DVE side; separate tiles so the dep
        #      tracker lets Act/DVE halves run in parallel) ----
        pz1 = [pp.tile([P, HB], F32, tag=f"pz1{s}", name=f"pz1{s}") for s in "LR"]
        pz2 = [pp.tile([P, HB], F32, tag=f"pz2{s}", name=f"pz2{s}") for s in "LR"]
        pz3 = [pp.tile([P, HB], F32, tag=f"pz3{s}", name=f"pz3{s}") for s in "LR"]
        pth = pp.tile([P, NJ, 2 * D], F32, tag="pth", name="pth")
        pmisc = pp.tile([P, 256, 2], F32, tag="pmisc", name="pmisc")
        # pmisc: [:, 0:4, :] pfr (j, mu/ls); partition0 cols 4:132 = xiT
        # chunks (4 x 128 fp16); [0:32, 132:164/164:196] xB-transpose ping/pong

        # ---- SBUF ----
        w0 = cp.tile([D, H], MMDT, tag="w0")
        w0r = cp.tile([1, D, H], MMDT, tag="w0r")
        wh0 = cp.tile([P, NB, H], MMDT, tag="wh0")
        wh1 = cp.tile([P, NB, H], MMDT, tag="wh1")
        wo = cp.tile([P, NB, 2 * D], MMDT, tag="wo")
        eye = cp.tile([P, P], MMDT, tag="eye")
        b0s = cp.tile([P, NB], F32, tag="b0s")
        bh0s = cp.tile([P, NB], F32, tag="bh0s")
        bh1s = cp.tile([P, NB], F32, tag="bh1s")
        zB = cp.tile([P, NJ, D], MMDT, tag="zB")
        xB = cp.tile([P, NJ, D], MMDT, tag="xB")
        xBf = cp.tile([P, NJ * D], F32, tag="xBf")
        xT4 = cp.tile([D, NJ, P], MMDT, tag="xT4")
        thetaS = cp.tile([P, NJ, 2 * D], F32, tag="thetaS")
        bml = cp.tile([P, NJ, 2 * D], F32, tag="bml")
        xiB = cp.tile([P, NJ], MMDT, tag="xiB")
        xiT = cp.tile([1, NJ, P], MMDT, tag="xiT")
        u8 = cp.tile([P, NJ, 2], MMDT, tag="u8")
        es = cp.tile([P, NJ], MMDT, tag="es")
        t2 = cp.tile([P, NJ], MMDT, tag="t2")
        aL = [[cp.tile([P, HB], MMDT, tag=f"a{l}L{r}", name=f"a{l}L{r}")
               for r in range(NB)] for l in range(3)]
        aR = [[cp.tile([P, HB], MMDT, tag=f"a{l}R{r}", name=f"a{l}R{r}")
               for r in range(NB)] for l in range(3)]
        S2L = cp.tile([P, HB], MMDT, tag="S2L")
        S2R = cp.tile([P, HB], MMDT, tag="S2R")
        S3L = cp.tile([P, HB], MMDT, tag="S3L")
        S3R = cp.tile([P, HB], MMDT, tag="S3R")
        z1nL = cp.tile([P, HB], MMDT, tag="z1nL")
        z1nR = cp.tile([P, HB], MMDT, tag="z1nR")

        # ---- DMA in ----
        nc.sync.dma_start(zB[:], d_z)
        nc.sync.dma_start(bml[:], d_bml)
        nc.sync.dma_start(thetaS[:], d_bml)      # theta starts as pure bias
        nc.sync.dma_start(b0s[:], d_b0)
        nc.sync.dma_start(bh0s[:], d_bh0)
        nc.sync.dma_start(bh1s[:], d_bh1)
        nc.sync.dma_start(w0[:], d_w0)
        nc.sync.dma_start(w0r[:], d_w0r)
        nc.sync.dma_start(eye[:], d_eye)
        nc.sync.dma_start(wo[:], d_wo)
        for c in range(NB):
            nc.sync.dma_start(wh0[:, c, :], d_wh0[:, c, :])
            nc.sync.dma_start(wh1[:, c, :], d_wh1[:, c, :])

        nc.vector.memset(xB[:], 0.0)

        xiTv = [pmisc[0:1, 4 + 32 * j:36 + 32 * j, :].bitcast(MMDT)
                for j in range(NJ)]
        xiTfull = pmisc[0:1, 4:132, :].bitcast(MMDT)         # [1,128,4] = 512
        xtt = [pmisc[0:D, 132:164, :].bitcast(MMDT),
               pmisc[0:D, 164:196, :].bitcast(MMDT)]

        def mm(out, lhsT, rhs, start, stop):
            nc.tensor.matmul(out, lhsT, rhs, start=start, stop=stop,
                             skip_group_check=True)

        def relu1(b):
            nc.scalar.activation(aL[0][b][:], pz1[0][:], ActFn.Relu,
                                 bias=b0s[:, b:b + 1], scale=1.0)
            nc.vector.tensor_scalar(aR[0][b][:], pz1[1][:],
                                    b0s[:, b:b + 1], 0.0, AluOp.add, AluOp.max)

        def relu2(b):
            nc.scalar.activation(aL[1][b][:], pz2[0][:], ActFn.Relu,
                                 bias=bh0s[:, b:b + 1], scale=1.0)
            nc.vector.tensor_scalar(aR[1][b][:], pz2[1][:],
                                    bh0s[:, b:b + 1], 0.0, AluOp.add, AluOp.max)

        def relu3(b):
            nc.scalar.activation(aL[2][b][:], pz3[0][:], ActFn.Relu,
                                 bias=bh1s[:, b:b + 1], scale=1.0)
            nc.vector.tensor_scalar(aR[2][b][:], pz3[1][:],
                                    bh1s[:, b:b + 1], 0.0, AluOp.add, AluOp.max)

        def xi_transpose():
            for j in range(NJ):
                nc.tensor.transpose(xiTv[j], xiB[:, j:j + 1], eye[:])
            nc.vector.tensor_scalar_add(xiT[:], xiTfull, 0.0)

        def layer_mms(pz, wh, a_in, b, cols, use_S, Ssb):
            """Accumulate one hidden layer for block b into pz (L and R)."""
            if use_S:
                todo = [c for c in cols if c >= b]
                for side in range(2):
                    mm(pz[side][:], eye[:], Ssb[side][:], True, False)
                    for k, c in enumerate(todo):
                        mm(pz[side][:], wh[:, c, b * P:(b + 1) * P],
                           a_in[side][c][:], False, k == len(todo) - 1)
            else:
                for side in range(2):
                    for k, c in enumerate(cols):
                        mm(pz[side][:], wh[:, c, b * P:(b + 1) * P],
                           a_in[side][c][:], k == 0, k == len(cols) - 1)

        # ---- step 0: x_0 = z_0 * exp(bout[D]) + bout[0] ----
        s0 = float(math.exp(bout[D]))
        m0c = float(bout[0])
        nc.vector.tensor_scalar(xiB[:], zB[:, :, 0], s0, m0c,
                                AluOp.mult, AluOp.add)
        nc.gpsimd.tensor_scalar_add(xB[:, :, 0], xiB[:], 0.0)
        xi_transpose()

        S2ready = [False] * NB
        theta_init = [False] * NJ

        for idx in range(1, STOP):
            act_blocks = active_at(idx)
            b_old = act_blocks[0]
            b_new = act_blocks[1] if len(act_blocks) > 1 else None
            ent = [b for b in act_blocks if entry[b] == idx]
            finalizing = [b for b in act_blocks if final[b] == idx]

            # rank-1 z1 for the persisted block (not for entering block;
            # block 0 "enters" at step 1 with a plain start=True rank-1)
            first = (b_old == 0 and idx == 1)
            if entry[b_old] != idx or first:
                mm(pz1[0][:], w0r[0:1, idx - 1, b_old * P:(b_old + 1) * P],
                   xiT[0:1, 0:NJ // 2, :], first, True)
                mm(pz1[1][:], w0r[0:1, idx - 1, b_old * P:(b_old + 1) * P],
                   xiT[0:1, NJ // 2:NJ, :], first, True)
            relu1(b_old)

            # entering block: overwrite pz1 after old relu1 read
            if b_new is not None:
                mm(pz1[0][:], eye[:], z1nL[:], True, False)
                mm(pz1[0][:], w0r[0:1, idx - 1, b_new * P:(b_new + 1) * P],
                   xiT[0:1, 0:NJ // 2, :], False, True)
                mm(pz1[1][:], eye[:], z1nR[:], True, False)
                mm(pz1[1][:], w0r[0:1, idx - 1, b_new * P:(b_new + 1) * P],
                   xiT[0:1, NJ // 2:NJ, :], False, True)
                relu1(b_new)

            # -- layer 2 --
            a1 = (aL[0], aR[0])
            a2 = (aL[1], aR[1])
            a3 = (aL[2], aR[2])
            cols2_old = [c for c in range(NB)
                         if nzh0[b_old, c] and g_lo[c] <= idx - 1]
            layer_mms(pz2, wh0, a1, b_old, cols2_old,
                      S2ready[b_old], (S2L, S2R))
            relu2(b_old)
            if b_new is not None:
                cols2_new = [c for c in range(NB)
                             if nzh0[b_new, c] and g_lo[c] <= idx - 1]
                layer_mms(pz2, wh0, a1, b_new, cols2_new, False, None)
                relu2(b_new)

            # -- layer 3 (old block may need a2[b_new]: emitted after) --
            cols3_old = [c for c in range(NB)
                         if nzh1[b_old, c] and g_lo[c] <= idx - 1]
            layer_mms(pz3, wh1, a2, b_old, cols3_old,
                      S2ready[b_old], (S3L, S3R))
            relu3(b_old)
            if b_new is not None:
                cols3_new = [c for c in range(NB)
                             if nzh1[b_new, c] and g_lo[c] <= idx - 1]
                layer_mms(pz3, wh1, a2, b_new, cols3_new, False, None)
                relu3(b_new)

            # -- frontier output contribution (batch-major, N=2) --
            for j in range(NJ):
                side, jj = (0, j) if j < NJ // 2 else (1, j - NJ // 2)
                for k, b in enumerate(act_blocks):
                    mm(pmisc[:, j, 0:2],
                       a3[side][b][:, jj * P:(jj + 1) * P],
                       wo[:, b, 2 * idx:2 * idx + 2],
                       k == 0, k == len(act_blocks) - 1)

            # -- tail --
            nc.vector.tensor_tensor(u8[:], pmisc[:, 0:NJ, :],
                                    thetaS[:, :, 2 * idx:2 * idx + 2],
                                    AluOp.add)
            nc.scalar.activation(es[:], u8[:, :, 1], ActFn.Exp)
            nc.gpsimd.tensor_tensor(t2[:], es[:], zB[:, :, idx], AluOp.mult)
            nc.gpsimd.tensor_tensor(xiB[:], t2[:], u8[:, :, 0], AluOp.add)
            nc.gpsimd.tensor_scalar_add(xB[:, :, idx], xiB[:], 0.0)
            if idx < STOP - 1:
                xi_transpose()

            # -- finalize theta (after tail read of pmisc/thetaS) --
            for b in finalizing:
                if idx >= STOP - 1:
                    continue
                for j in range(NJ):
                    side, jj = (0, j) if j < NJ // 2 else (1, j - NJ // 2)
                    mm(pth[:, j, :],
                       a3[side][b][:, jj * P:(jj + 1) * P],
                       wo[:, b, :],
                       not theta_init[j], True)
                    theta_init[j] = True
                nc.vector.tensor_tensor(thetaS[:], pth[:], bml[:], AluOp.add)

            # -- cache S2/S3 one step after entry (pz2/pz3 as scratch) --
            b = b_old
            if b > 0 and not S2ready[b] and idx == entry[b] + 1 \
                    and idx < final[b]:
                cc2 = [c for c in range(NB) if nzh0[b, c] and c < b]
                cc3 = [c for c in range(NB) if nzh1[b, c] and c < b]
                for side in range(2):
                    for k, c in enumerate(cc2):
                        mm(pz2[side][:], wh0[:, c, b * P:(b + 1) * P],
                           a1[side][c][:], k == 0, k == len(cc2) - 1)
                    for k, c in enumerate(cc3):
                        mm(pz3[side][:], wh1[:, c, b * P:(b + 1) * P],
                           a2[side][c][:], k == 0, k == len(cc3) - 1)
                nc.scalar.copy(S2L[:], pz2[0][:])
                nc.vector.tensor_scalar_add(S2R[:], pz2[1][:], 0.0)
                nc.scalar.copy(S3L[:], pz3[0][:])
                nc.vector.tensor_scalar_add(S3R[:], pz3[1][:], 0.0)
                S2ready[b] = True
            if final[b_old] == idx:
                S2ready[b_old] = False

            # -- prefetch z1 for the block entering next step --
            pre = [bb for bb in range(1, NB) if entry[bb] == idx + 1]
            for bb in pre:
                for j in range(NJ):
                    pg = xtt[j % 2]
                    nc.tensor.transpose(pg, xB[:, j, :], eye[:])
                    nc.vector.tensor_scalar_add(xT4[:, j, :], pg, 0.0)
                mm(pz2[0][:], w0[:, bb * P:(bb + 1) * P],
                   xT4[:, 0:NJ // 2, :], True, True)
                mm(pz2[1][:], w0[:, bb * P:(bb + 1) * P],
                   xT4[:, NJ // 2:NJ, :], True, True)
                nc.scalar.copy(z1nL[:], pz2[0][:])
                nc.vector.tensor_scalar_add(z1nR[:], pz2[1][:], 0.0)

        # ---- output ----
        nc.scalar.copy(xBf[:], xB[:])
        nc.sync.dma_start(d_out, xBf[:])

    nc.compile()
    return nc


_CACHE = {}


def _get_program(prep):
    key = (DTYPE, STOP, hashlib.md5(prep["bout"].tobytes()).hexdigest())
    if key not in _CACHE:
        _CACHE[key] = _build(prep)
    return _CACHE[key]


def _run(inputs, trace=False):
    z = np.asarray(inputs["z"], dtype=np.float32)
    prep = _host_prep(np.asarray(inputs["W0"], np.float32),
                      np.asarray(inputs["b0"], np.float32),
                      np.asarray(inputs["Wh"], np.float32),
                      np.asarray(inputs["bh"], np.float32),
                      np.asarray(inputs["Wout"], np.float32),
                      np.asarray(inputs["bout"], np.float32))
    nc = _get_program(prep)

    eye = np.eye(P, dtype=NPDT)
    bout32 = prep["bout"].astype(np.float32)
    bml1 = np.empty(2 * D, dtype=np.float32)
    bml1[0::2] = bout32[:D]
    bml1[1::2] = bout32[D:]
    bml = np.ascontiguousarray(
        np.broadcast_to(np.tile(bml1, NJ), (P, NJ * 2 * D))).astype(np.float32)
    in_maps = []
    for c in range(NCORES):
        zs = z[c * BC:(c + 1) * BC, :]                     # [512, 32]
        # batch-major: [p, j, i] = z[j*128+p, i]
        zb = np.ascontiguousarray(
            zs.reshape(NJ, P, D).transpose(1, 0, 2).reshape(P, NJ * D)
        ).astype(NPDT)
        in_maps.append({
            "w0t": prep["W0T"], "wh0t": prep["Wh0T"], "wh1t": prep["Wh1T"],
            "wot": prep["WoIT"], "b0l": prep["b0L"], "bh0l": prep["bh0L"],
            "bh1l": prep["bh1L"], "zb": zb, "eye": eye,
            "w0r": prep["W0T"].reshape(1, D, H), "bml": bml,
        })

    res = run_bass_kernel_spmd(nc, in_maps, core_ids=list(range(NCORES)),
                               trace=trace)
    out = np.empty((B, D), dtype=np.float32)
    for c in range(NCORES):
        buf = res.results[c]["out"]                        # [128, 128]
        out[c * BC:(c + 1) * BC, :] = (
            buf.reshape(P, NJ, D).transpose(1, 0, 2).reshape(BC, D))
    return out, res


def kernel(**inputs):
    out, _ = _run(inputs, trace=False)
    return out
